# revision 1
# baseline (speedup 1.0000x reference)
"""Trainium2 Bass kernel for a CrossAttentionBlock.

Reference computation (B=4, C=256, H=W=64, 4 heads, head_dim=64):
  q = Wq @ GN(x);  k = Wk @ GN(ctx);  v = Wv @ ctx        (1x1 convs)
  attn = softmax(q^T k / sqrt(hd))  per (batch, head)
  out  = x + Wo @ (v @ attn^T) + bo

Sharding: 8 cores = (batch b = core//2) x (spatial half s = core%2); each
core computes its [256, 2048] output chunk; k/v span the full 4096 columns.

Main optimizations over a straightforward implementation:
  - bf16 host inputs/outputs; x is shipped with the core's own spatial
    half first (GN stats are permutation-invariant over columns), so no
    separate residual input is needed.
  - GroupNorm folded into the projection weights on-chip: Wk' = Wk.diag(a)
    (the K-side bias adds a per-column constant to S that cancels in
    softmax and is dropped); Wq' = Wq.diag(a) with bias qb = Wq @ d folded
    into q's PSUM->SBUF copy. No normalized activation copies exist.
  - q/k quantized to fp8e4m3 at unit variance (1/sqrt(hd) moved into the
    exp scale) and remapped once by DMA into a pair-interleaved layout so
    every S^T matmul runs in fp8 DoubleRow mode (half cost). exp writes
    fp8e5m2 pair tiles consumed by DoubleRow attn@v matmuls (half cost);
    e5m2 because max S/sqrt(hd) is ~7.3 so exp reaches ~1400 > e4m3 max.
  - softmax exp split across two engines: exact exp on ScalarE; a one-op
    Schraudolph exp2 on DVE (round(4*log2e*S/8 + B) written as uint8 into
    an fp8e5-viewed tile). The approximation's systematic bias cancels in
    softmax; the random part averages out over the softmax support
    (~1e-3 added relative error). This halves the exp wall-clock.
  - v^T carries 64 ones-columns, so the attn@v matmul emits the softmax
    denominator pre-replicated across 64 partitions: the divide is just a
    DVE reciprocal + multiply.
  - One flat software pipeline over (d-chunk, head) with a 3-deep S^T PSUM
    ring (hides exp latency from PE); V^T/K projections and Wo output
    chunks are injected into the early/boundary slots of the pipeline.
  - build_module(loop=K) emits the body K times for on-device loop timing:
    the marginal iteration cost is the true HW exec time with all
    dispatch/transfer overhead differenced out.
"""

import sys

if "/opt/trn_rl_repo" not in sys.path:
    sys.path.insert(0, "/opt/trn_rl_repo")

import copy
from contextlib import ExitStack

import numpy as np
import ml_dtypes

import bass_rust
import concourse.bass as bass
import concourse.mybir as mybir
import concourse.tile as tile
from concourse.bass_utils import run_bass_kernel_spmd
from concourse.vector_clock import ScopedClock

BF16 = ml_dtypes.bfloat16
F32 = mybir.dt.float32
BF = mybir.dt.bfloat16
F8 = mybir.dt.float8e5
F8E4 = mybir.dt.float8e4

N_CORES = 8
B, C, HW = 4, 256, 4096
HALF = HW // 2          # spatial columns per core
NH, HD = 4, 64          # heads, head dim
P = 128                 # partitions
NSUB = C // P           # channel subtiles (2)
GROUPS = 32             # groupnorm groups (16 per channel-subtile)
CH_PER_G = C // GROUPS  # 8
GN_N = CH_PER_G * HW    # elements per group (32768)
EPS = 1e-5
DJ = 1024               # main-loop d-chunk (exp granularity)
NDJ = HALF // DJ        # 2
NE = HW // P            # 32 e-chunks
ALU = mybir.AluOpType
ACTF = mybir.ActivationFunctionType
AXX = mybir.AxisListType.X

SPLIT_FOR_WALRUS = True  # sim_check.py disables: CoreSim rejects raw SyncInfo


class SplitDrainTileContext(tile.TileContext):
    """TileContext whose exit drain splits sem waits across multiple Drain
    instructions — the walrus build in this container rejects >2 sync waits
    on a single Drain ("Too many sync wait commands")."""

    def _drain_and_barrier(self, tick_clock, wait_clock):
        if not SPLIT_FOR_WALRUS:
            return super()._drain_and_barrier(tick_clock, wait_clock)
        drain_inst = self.nc.sync.drain()
        wait_clock.add_sem_waits(
            drain_inst.ins, ScopedClock({None: tick_clock.global_clock})
        )
        si = drain_inst.ins.sync_info
        if si is not None and si.on_wait and len(si.on_wait) > 1:
            waits = list(si.on_wait)
            si.on_wait = waits[:1]
            drain_inst.ins.sync_info = si
            for w in waits[1:]:
                extra = self.nc.sync.drain()
                extra.ins.sync_info = bass_rust.SyncInfo(on_wait=[w], on_update=[])
        self.nc.all_engine_barrier()
        popped = self.nc._tile_sem_poison_stack.pop()
        assert popped is self._sem_poison
        self.nc.clear_and_free_semaphores(list(self.sems.allocated().values()))
        self.nc.all_engine_barrier()


_NOP_TMPL = []


def _nop_template():
    if not _NOP_TMPL:
        tb = bass.Bass()
        with tb.bb("t"):
            _NOP_TMPL.append(copy.copy(tb.vector.nop().ins))
    return _NOP_TMPL[0]


def _split_excess_waits(nc, limit=1):
    """This container's walrus rejects instructions carrying more than ~2
    sync-wait commands. Spill excess waits onto same-engine NoOps inserted
    just before the overloaded instruction (waiting earlier on the same
    engine is semantics-preserving; NoOps have no dependents, so no cycles
    can form)."""
    tmpl = _nop_template()
    n = 0

    def fix(blk):
        nonlocal n
        if hasattr(blk, "instructions"):
            out = []
            changed = False
            for inst in blk.instructions:
                si = inst.sync_info
                ow = list(si.on_wait) if (si is not None and si.on_wait) else []
                lim = 1 if ("DMA" in inst.opcode or inst.opcode == "Drain") \
                    else limit
                if len(ow) > lim:
                    changed = True
                    for w in ow[:-lim]:
                        sp = copy.copy(tmpl)
                        n += 1
                        sp.name = f"I-wsp-{n}"
                        sp.engine = inst.engine
                        sp.sync_info = bass_rust.SyncInfo(on_wait=[w],
                                                          on_update=[])
                        out.append(sp)
                    si.on_wait = ow[-lim:]
                    inst.sync_info = si
                out.append(inst)
            if changed:
                blk.instructions = out
        for sub in getattr(blk, "blocks", []) or []:
            fix(sub)

    for f in nc.m.functions:
        for blk in f.blocks:
            fix(blk)
    return n


def build_module(loop: int = 1) -> bass.Bass:
    """loop > 1 emits the kernel body that many times back-to-back (each in
    its own tile context, so semaphores reset between iterations). Used for
    on-device loop timing: the marginal cost of +1 iteration is the true HW
    exec time, with all per-call dispatch/transfer overhead differenced out."""
    nc = bass.Bass()

    dr = {}
    dr["xb"] = nc.dram_tensor("xb", [C, HW], BF, kind="ExternalInput")
    dr["cb"] = nc.dram_tensor("cb", [C, HW], BF, kind="ExternalInput")
    dr["wqt"] = nc.dram_tensor("wqt", [C, C], BF, kind="ExternalInput")
    dr["wkt"] = nc.dram_tensor("wkt", [C, C], BF, kind="ExternalInput")
    dr["wvt"] = nc.dram_tensor("wvt", [C, C], BF, kind="ExternalInput")
    dr["wot"] = nc.dram_tensor("wot", [C, C], BF, kind="ExternalInput")
    dr["gnp"] = nc.dram_tensor("gnp", [P, NSUB, 4], F32, kind="ExternalInput")
    dr["bo"] = nc.dram_tensor("bo", [P, NSUB], F32, kind="ExternalInput")
    dr["gsel"] = nc.dram_tensor("gsel", [P, 16], BF, kind="ExternalInput")
    dr["selt"] = nc.dram_tensor("selt", [16, P], F32, kind="ExternalInput")
    dr["out"] = nc.dram_tensor("out", [C, HALF], BF, kind="ExternalOutput")

    for _ in range(loop):
        with SplitDrainTileContext(nc) as tc:
            _emit(nc, tc, dr)
    if SPLIT_FOR_WALRUS:
        _split_excess_waits(nc)
    return nc


def _emit(nc, tc, dr):
    with ExitStack() as ctx:
        pw = ctx.enter_context(tc.tile_pool(name="pw", bufs=1))
        pmain = ctx.enter_context(tc.tile_pool(name="pmain", bufs=1))
        ptp = ctx.enter_context(tc.tile_pool(name="ptp", bufs=4))
        psmall = ctx.enter_context(tc.tile_pool(name="psmall", bufs=2))

        # ---- tiles ----
        wq_sb = pw.tile([P, NSUB, C], BF, name="wq_sb")
        wk_sb = pw.tile([P, NSUB, C], BF, name="wk_sb")
        wv_sb = pw.tile([P, NSUB, C], BF, name="wv_sb")
        wo_sb = pw.tile([P, NSUB, C], BF, name="wo_sb")
        gnp_sb = pw.tile([P, NSUB, 4], F32, name="gnp_sb")
        bo_sb = pw.tile([P, NSUB], F32, name="bo_sb")
        gsel_sb = pw.tile([P, 16], BF, name="gsel_sb")
        selt_sb = pw.tile([16, P], F32, name="selt_sb")
        eps_sb = pw.tile([16, 1], F32, name="eps_sb")
        nc.vector.memset(eps_sb[:], EPS)

        xb_sb = pmain.tile([P, NSUB, HW], BF, name="xb_sb")
        cb_sb = pmain.tile([P, NSUB, HW], BF, name="cb_sb")
        q_sb = pmain.tile([P, NSUB, HALF], F8E4, name="q_sb")
        k_sb = pmain.tile([P, NSUB, HW], F8E4, name="k_sb")
        q8_sb = [pmain.tile([HD, 2, HALF], F8E4, name=f"q8_sb{i}")
                 for i in range(NSUB)]
        k8_sb = [pmain.tile([HD, 2, HW], F8E4, name=f"k8_sb{i}")
                 for i in range(NSUB)]
        vt8_sb = pmain.tile([P, NE // 2, 2, NH, 2 * HD], F8, name="vt8_sb")
        ao_sb = pmain.tile([P, NSUB, HALF], BF, name="ao_sb")
        stats_sb = pmain.tile([16, 8], F32, name="stats_sb")
        wqs_sb = pmain.tile([P, NSUB, C], BF, name="wqs_sb")
        wks_sb = pmain.tile([P, NSUB, C], BF, name="wks_sb")
        qb_sb = pmain.tile([P, NSUB], F32, name="qb_sb")

        # ones-columns of v^T (softmax denominator rows, pre-replicated);
        # on gpsimd to keep DVE free for the stats chain
        nc.gpsimd.memset(vt8_sb[:, :, :, :, HD:2 * HD], 1.0)

        # ---- input DMAs, ordered by first use: the SP queue is serial, so
        # context (stats start immediately) and x go before the weights that
        # are only needed later in the prep chain ----
        xv = dr["xb"][:].rearrange("(t p) d -> p t d", p=P)
        cv = dr["cb"][:].rearrange("(t p) d -> p t d", p=P)
        NJ = 4
        CHK = HW // NJ  # 1024
        nc.sync.dma_start(gsel_sb[:], dr["gsel"][:])
        for j in range(NJ):
            sl = slice(j * CHK, (j + 1) * CHK)
            nc.sync.dma_start(cb_sb[:, :, sl], cv[:, :, sl])
        for j in range(NJ):
            sl = slice(j * CHK, (j + 1) * CHK)
            nc.sync.dma_start(xb_sb[:, :, sl], xv[:, :, sl])
        nc.sync.dma_start(selt_sb[:], dr["selt"][:])
        nc.sync.dma_start(gnp_sb[:], dr["gnp"][:])
        nc.sync.dma_start(
            wk_sb[:], dr["wkt"][:].rearrange("(t p) o -> p t o", p=P))
        nc.sync.dma_start(
            wq_sb[:], dr["wqt"][:].rearrange("(t p) o -> p t o", p=P))
        nc.sync.dma_start(
            wv_sb[:], dr["wvt"][:].rearrange("(t p) o -> p t o", p=P))
        nc.sync.dma_start(
            wo_sb[:], dr["wot"][:].rearrange("(t p) o -> p t o", p=P))
        nc.sync.dma_start(bo_sb[:], dr["bo"][:])

        # ============ prep: GN stats -> folded weights -> Q/K/V^T ==========
        with ExitStack() as prep:
            pps = prep.enter_context(
                tc.tile_pool(name="pps", bufs=4, space="PSUM"))
            pmm = prep.enter_context(
                tc.tile_pool(name="pmm", bufs=1, space="PSUM"))
            pchunk = prep.enter_context(tc.tile_pool(name="pchunk", bufs=2))

            def stats_mms(src_sb, tensor_idx, sq_eng=None):
                """Per-group sums of src and src^2 via the selector matmul,
                streamed in NJ chunks."""
                ps = {
                    (t, kind): pps.tile([16, 512], F32, tag="stat",
                                        name=f"ps_st{tensor_idx}{t}{kind}")
                    for t in range(NSUB) for kind in range(2)
                }
                for j in range(NJ):
                    sl = slice(j * CHK, (j + 1) * CHK)
                    sq = pchunk.tile([P, NSUB, CHK], BF, tag="sq", name="sq")
                    sq_eng.tensor_mul(sq[:], src_sb[:, :, sl],
                                      src_sb[:, :, sl])
                    for t in range(NSUB):
                        for half in range(CHK // 512):
                            s2 = slice(j * CHK + half * 512,
                                       j * CHK + (half + 1) * 512)
                            s2q = slice(half * 512, (half + 1) * 512)
                            first = j == 0 and half == 0
                            last = j == NJ - 1 and half == CHK // 512 - 1
                            nc.tensor.matmul(ps[(t, 0)][:], gsel_sb[:],
                                             src_sb[:, t, s2],
                                             start=first, stop=last)
                            nc.tensor.matmul(ps[(t, 1)][:], gsel_sb[:],
                                             sq[:, t, s2q],
                                             start=first, stop=last)
                for t in range(NSUB):
                    for kind in range(2):
                        nc.vector.reduce_sum(
                            stats_sb[:, 4 * tensor_idx + 2 * t + kind:
                                     4 * tensor_idx + 2 * t + kind + 1],
                            ps[(t, kind)][:], axis=AXX)

            def affine(tensor_idx, grp_out, packed):
                """stats columns -> per-group (mean, rstd) packed [16, 4],
                expanded to per-channel [P, 4] via the fp32 selector."""
                inv_n = 1.0 / GN_N
                for t in range(NSUB):
                    col = 4 * tensor_idx + 2 * t
                    mean = packed[:, 2 * t:2 * t + 1]
                    rstd = packed[:, 2 * t + 1:2 * t + 2]
                    nc.vector.tensor_scalar_mul(
                        mean, stats_sb[:, col:col + 1], inv_n)
                    nc.vector.tensor_scalar_mul(
                        rstd, stats_sb[:, col + 1:col + 2], inv_n)
                    m2 = psmall.tile([16, 1], F32, tag="m2", name="m2")
                    nc.vector.tensor_mul(m2[:], mean, mean)
                    nc.vector.tensor_sub(rstd, rstd, m2[:])
                    nc.scalar.activation(rstd, rstd, ACTF.Sqrt, bias=eps_sb[:])
                    nc.vector.reciprocal(rstd, rstd)
                psg = pmm.tile([P, 512], F32, tag="mm", name="psg")
                nc.tensor.matmul(psg[:, :4], selt_sb[:], packed[:],
                                 start=True, stop=True)
                nc.vector.tensor_copy(grp_out[:], psg[:, :4])

            stats_mms(cb_sb, 0, sq_eng=nc.vector)
            stats_mms(xb_sb, 1, sq_eng=nc.gpsimd)

            packed_c = psmall.tile([16, 4], F32, tag="pk", name="packed_c")
            grp_c = psmall.tile([P, 4], F32, tag="gr", name="grp_c")
            affine(0, grp_c, packed_c)
            a_c = psmall.tile([P, NSUB], F32, tag="ac", name="a_c")
            for t in range(NSUB):
                nc.vector.tensor_mul(a_c[:, t:t + 1], gnp_sb[:, t, 2:3],
                                     grp_c[:, 2 * t + 1:2 * t + 2])
                nc.vector.tensor_scalar(
                    wks_sb[:, t], wk_sb[:, t], a_c[:, t:t + 1], None,
                    op0=ALU.mult)
            packed_x = psmall.tile([16, 4], F32, tag="pk", name="packed_x")
            grp_x = psmall.tile([P, 4], F32, tag="gr", name="grp_x")
            affine(1, grp_x, packed_x)
            a_x = psmall.tile([P, NSUB], F32, tag="ac", name="a_x")
            d_x = psmall.tile([P, NSUB], BF, tag="dx", name="d_x")
            for t in range(NSUB):
                nc.vector.tensor_mul(a_x[:, t:t + 1], gnp_sb[:, t, 0:1],
                                     grp_x[:, 2 * t + 1:2 * t + 2])
                tmp = psmall.tile([P, 1], F32, tag="tmp", name="tmp")
                nc.vector.tensor_mul(tmp[:], grp_x[:, 2 * t:2 * t + 1],
                                     a_x[:, t:t + 1])
                nc.vector.tensor_scalar(
                    d_x[:, t:t + 1], gnp_sb[:, t, 1:2], tmp[:, 0:1], None,
                    op0=ALU.subtract)
                nc.vector.tensor_scalar(
                    wqs_sb[:, t], wq_sb[:, t], a_x[:, t:t + 1], None,
                    op0=ALU.mult)

        # ================= attention main pipeline =================
        pst = ctx.enter_context(tc.tile_pool(name="psum_st", bufs=3,
                                             space="PSUM"))
        pout = ctx.enter_context(tc.tile_pool(name="psum_out", bufs=1,
                                              space="PSUM"))

        def borrow():
            return pst.tile([P, DJ], F32, tag="st", name="pp")

        def k_proj(i, copy_engine):
            for jd in range(HW // 512):
                psk = borrow()
                for t in range(NSUB):
                    nc.tensor.matmul(
                        psk[:, :512], wks_sb[:, t, i * P:(i + 1) * P],
                        cb_sb[:, t, jd * 512:(jd + 1) * 512],
                        start=(t == 0), stop=(t == NSUB - 1))
                copy_engine.tensor_copy(
                    k_sb[:, i, jd * 512:(jd + 1) * 512], psk[:, :512])

        def vt_proj(ec, on_scalar):
            psv = borrow()
            for t in range(NSUB):
                nc.tensor.matmul(
                    psv[:, :C], cb_sb[:, t, ec * P:(ec + 1) * P],
                    wv_sb[:, t, :],
                    start=(t == 0), stop=(t == NSUB - 1))
            dst = vt8_sb[:, ec // 2, ec % 2, :, 0:HD]
            srcv = psv[:, :C].rearrange("p (h c) -> p h c", c=HD)
            if on_scalar:
                nc.scalar.copy(dst, srcv)
            else:
                nc.vector.tensor_copy(dst, srcv)

        k_proj(0, nc.vector)
        psb = borrow()
        for i in range(NSUB):
            for t in range(NSUB):
                nc.tensor.matmul(psb[:, 256 * i:256 * i + 1],
                                 wq_sb[:, t, i * P:(i + 1) * P],
                                 d_x[:, t:t + 1],
                                 start=(t == 0), stop=(t == NSUB - 1))
        nc.vector.tensor_copy(qb_sb[:, 0:1], psb[:, 0:1])
        nc.vector.tensor_copy(qb_sb[:, 1:2], psb[:, 256:257])
        # partition remap (channel c -> partition 32h' + c//2, slot c%2)
        # for DoubleRow: SBUF->SBUF DMA, element orders match exactly
        for i in range(NSUB):
            for jd in range(HALF // 512):
                psq = borrow()
                for t in range(NSUB):
                    nc.tensor.matmul(
                        psq[:, :512], wqs_sb[:, t, i * P:(i + 1) * P],
                        xb_sb[:, t, jd * 512:(jd + 1) * 512],
                        start=(t == 0), stop=(t == NSUB - 1))
                nc.vector.tensor_scalar(
                    q_sb[:, i, jd * 512:(jd + 1) * 512], psq[:, :512],
                    qb_sb[:, i:i + 1], None, op0=ALU.add)
            nc.sync.dma_start(q8_sb[i][:], q_sb[:, i, :])
            if i == 0:
                nc.sync.dma_start(k8_sb[0][:, :, :HALF],
                                  k_sb[:, 0, :HALF])
                nc.sync.dma_start(k8_sb[0][:, :, HALF:],
                                  k_sb[:, 0, HALF:])
        vt_proj(0, True)
        vt_proj(1, True)
        vt_proj(2, True)

        items = [(dj, h) for dj in range(NDJ) for h in range(NH)]
        state = {}   # per-item: po tile, pts list
        wo_queue = []

        # exp work is split between ScalarE and DVE, at e-chunk-PAIR
        # granularity. "S" pairs: exact exp on ScalarE, written as fp8e5
        # into a pair-interleaved tile, consumed by a half-cost DoubleRow
        # out-matmul (the pair dim supplies the second contraction row).
        # "D" pairs: one-op Schraudolph exp2 on DVE (bf16 bit trick: write
        # round(S*128*log2e + B) as int16 into a bf16-viewed tile; +-3%
        # element error whose systematic part cancels in softmax and whose
        # random part averages out over the softmax support), consumed by
        # normal bf16 out-matmuls. fp8e5 (not e4) because max S is ~7.2
        # so exp reaches ~1400 > e4m3's 448.
        # e5m2 bit trick for the DVE leg: bits = round(4*log2e*S + B) as
        # uint8 viewed as fp8e5. For this data |S| <= 7.3 so bits stay in
        # [18, 102] -- far from both uint8 wrap and e5m2 inf (124).
        EXPA = 0.125 * 4.0 * 1.4426950408889634
        EXPB = 59.8
        NPAIR = NE // 2
        SS_PAIRS = (2, 6, 10, 14)  # both chunks on ScalarE; rest split S/D

        def emit_st_exp(n, ec):
            dj, h = items[n]
            pb = (h % 2) * HD
            hs = h // 2
            d0 = dj * DJ
            m, half = ec // 2, ec % 2
            st = pst.tile([P, DJ], F32, tag="st", name="st")
            hb = 32 * (h % 2)
            lhsT = k8_sb[hs][hb:hb + 32, :, ec * P:(ec + 1) * P]
            for s in range(DJ // 512):
                nc.tensor.matmul(
                    st[:, s * 512:(s + 1) * 512], lhsT,
                    q8_sb[hs][hb:hb + 32, :,
                              d0 + s * 512:d0 + (s + 1) * 512],
                    start=True, stop=True,
                    perf_mode=mybir.MatmulPerfMode.DoubleRow)
            if half == 0:
                pt = ptp.tile([P, 2, DJ], F8, tag="ptq", bufs=6, name="ptq")
                state[n]["pts"].append(pt)
            pt = state[n]["pts"][m]
            if half == 0 or m in SS_PAIRS:
                nc.scalar.activation(pt[:, half], st[:], ACTF.Exp,
                                     scale=0.125)
            else:
                nc.vector.tensor_scalar(pt[:, half].bitcast(mybir.dt.uint8),
                                        st[:], EXPA, EXPB,
                                        op0=ALU.mult, op1=ALU.add)

        def emit_out_pair(n, m):
            dj, h = items[n]
            po = state[n]["po"]
            pt = state[n]["pts"][m]
            vl = vt8_sb[:, m, :, h, :]
            for s in range(DJ // 512):
                nc.tensor.matmul(
                    po[:, s * 512:(s + 1) * 512], vl,
                    pt[:, :, s * 512:(s + 1) * 512],
                    start=(m == 0), stop=(m == NPAIR - 1),
                    perf_mode=mybir.MatmulPerfMode.DoubleRow)

        def emit_divide(n):
            dj, h = items[n]
            pb = (h % 2) * HD
            hs = h // 2
            d0 = dj * DJ
            po = state[n]["po"]
            rc = psmall.tile([HD, DJ], F32, tag="rc", name="rc")
            nc.vector.reciprocal(rc[:], po[HD:2 * HD, :])
            for s in range(DJ // 512):
                nc.vector.tensor_mul(
                    ao_sb[pb:pb + HD, hs, d0 + s * 512:d0 + (s + 1) * 512],
                    po[0:HD, s * 512:(s + 1) * 512],
                    rc[:, s * 512:(s + 1) * 512])
            if h == NH - 1:
                for i in range(NSUB):
                    for s in range(DJ // 512):
                        wo_queue.append((dj, i, s))

        def emit_wo_chunk():
            # Wo psum borrows a slot from the st pool (PSUM is fully booked:
            # st 2x2 + po 2x2 banks); the brief rotation wait is absorbed by
            # the exp engines' slack.
            dj, i, s = wo_queue.pop(0)
            sl = slice(dj * DJ + s * 512, dj * DJ + (s + 1) * 512)
            pso = pst.tile([P, DJ], F32, tag="st", name="pso")
            for t in range(NSUB):
                nc.tensor.matmul(
                    pso[:, :512], wo_sb[:, t, i * P:(i + 1) * P],
                    ao_sb[:, t, sl],
                    start=(t == 0), stop=(t == NSUB - 1))
            ot = psmall.tile([P, 512], BF, tag="ot", bufs=3, name="ot")
            nc.vector.scalar_tensor_tensor(
                ot[:], pso[:, :512], bo_sb[:, i:i + 1], xb_sb[:, i, sl],
                op0=ALU.add, op1=ALU.add)
            nc.sync.dma_start(
                dr["out"][:].rearrange("(t p) d -> p t d", p=P)[:, i, sl],
                ot[:])

        for n in range(len(items)):
            state[n] = {"po": None, "pts": []}
            for ec in range(NE):
                if ec == 0:
                    emit_st_exp(n, 0)
                    continue
                if ec == 1 and n > 0:
                    # finish the previous item BEFORE this head's second
                    # chunk: the divide lands ahead of exp(n,1) in DVE
                    # program order, releasing the single pout slot ~1.2us
                    # earlier so PE's first out-matmul does not stall
                    emit_out_pair(n - 1, NPAIR - 1)
                    emit_divide(n - 1)
                    state[n - 1] = None
                emit_st_exp(n, ec)
                if n == 0 and 1 <= ec < NE - 2:
                    vt_proj(ec + 2, on_scalar=bool(ec % 2))
                if n == 1 and ec < 16 and ec % 2 == 1:
                    jd = (ec - 1) // 2
                    psk = borrow()
                    for t in range(NSUB):
                        nc.tensor.matmul(
                            psk[:, :512], wks_sb[:, t, P:2 * P],
                            cb_sb[:, t, jd * 512:(jd + 1) * 512],
                            start=(t == 0), stop=(t == NSUB - 1))
                    if ec % 4:
                        nc.vector.tensor_copy(
                            k_sb[:, 1, jd * 512:(jd + 1) * 512],
                            psk[:, :512])
                    else:
                        nc.scalar.copy(
                            k_sb[:, 1, jd * 512:(jd + 1) * 512],
                            psk[:, :512])
                if n == 1 and ec == 17:
                    nc.sync.dma_start(k8_sb[1][:], k_sb[:, 1, :])
                if ec == 2:
                    state[n]["po"] = pout.tile([P, DJ], F32, tag="po",
                                               name="po")
                if ec >= 3 and ec % 2 == 1:
                    emit_out_pair(n, ec // 2 - 1)
                if wo_queue and ec % 4 == 0:
                    emit_wo_chunk()
        n = len(items) - 1
        emit_out_pair(n, NPAIR - 1)
        emit_divide(n)
        while wo_queue:
            emit_wo_chunk()


_CACHE = {}


def _get_module():
    if "nc" not in _CACHE:
        _CACHE["nc"] = build_module()
    return _CACHE["nc"]


def make_in_maps(inputs):
    x = np.ascontiguousarray(np.asarray(inputs["x"], np.float32).reshape(B, C, HW))
    cx = np.ascontiguousarray(
        np.asarray(inputs["context"], np.float32).reshape(B, C, HW))
    Wq = np.asarray(inputs["Wq"], np.float32)
    Wk = np.asarray(inputs["Wk"], np.float32)
    Wv = np.asarray(inputs["Wv"], np.float32)
    Wo = np.asarray(inputs["Wo"], np.float32)
    bo = np.asarray(inputs["bo"], np.float32)
    gq_w = np.asarray(inputs["gn_q_w"], np.float32)
    gq_b = np.asarray(inputs["gn_q_b"], np.float32)
    gc_w = np.asarray(inputs["gn_ctx_w"], np.float32)
    gc_b = np.asarray(inputs["gn_ctx_b"], np.float32)

    wqt = np.ascontiguousarray(Wq.T).astype(BF16)
    wkt = np.ascontiguousarray(Wk.T).astype(BF16)
    wvt = np.ascontiguousarray(Wv.T).astype(BF16)
    wot = np.ascontiguousarray(Wo.T).astype(BF16)
    # gnp columns: (w_x, b_x, w_ctx, b_ctx) per channel
    gnp = np.stack([gq_w, gq_b, gc_w, gc_b], axis=-1).reshape(NSUB, P, 4)
    gnp = np.ascontiguousarray(gnp.transpose(1, 0, 2))
    bo_t = np.ascontiguousarray(bo.reshape(NSUB, P).T)
    gsel = np.zeros((P, 16), BF16)
    for p in range(P):
        gsel[p, p // CH_PER_G] = 1
    selt = np.ascontiguousarray(gsel.astype(np.float32).T)

    xbf = x.astype(BF16)
    cbf = cx.astype(BF16)
    shared = dict(wqt=wqt, wkt=wkt, wvt=wvt, wot=wot, gnp=gnp, bo=bo_t,
                  gsel=gsel, selt=selt)
    in_maps = []
    for core in range(N_CORES):
        b, s = core // 2, core % 2
        m = dict(shared)
        # core's own spatial half first (stats are permutation-invariant)
        mine = xbf[b][:, s * HALF:(s + 1) * HALF]
        other = xbf[b][:, (1 - s) * HALF:(2 - s) * HALF]
        m["xb"] = np.ascontiguousarray(np.concatenate([mine, other], axis=1))
        m["cb"] = cbf[b]
        in_maps.append(m)
    return in_maps


def assemble(results):
    outf = np.empty((B, C, HW), np.float32)
    for core in range(N_CORES):
        b, s = core // 2, core % 2
        outf[b][:, s * HALF:(s + 1) * HALF] = np.asarray(
            results[core]["out"], np.float32)
    return outf.reshape(B, C, 64, 64)


def kernel(**inputs) -> np.ndarray:
    nc = _get_module()
    in_maps = make_in_maps(inputs)
    res = run_bass_kernel_spmd(nc, in_maps, core_ids=list(range(N_CORES)))
    return assemble(res.results)



# revision 2
# speedup vs baseline: 18.9727x; 18.9727x over previous
"""Trainium2 Bass kernel for a CrossAttentionBlock (v2).

Reference computation (B=4, C=256, H=W=64, 4 heads, head_dim=64):
  q = Wq @ GN(x);  k = Wk @ GN(ctx);  v = Wv @ ctx        (1x1 convs)
  attn = softmax(q^T k / sqrt(hd))  per (batch, head)
  out  = x + Wo @ (v @ attn^T) + bo

Sharding: 8 cores = (batch b = core//2) x (spatial half s = core%2); each
core computes its [256, 2048] output chunk; k/v span the full 4096 columns.

v2 changes over the v1 baseline (which simulated at 243us/iteration with
ScalarE+DVE ~172us busy each):
  - All projections (Q/K/V) and the GroupNorm statistics matmuls run in
    fp8e4m3 DoubleRow mode: x/ctx ship pair-interleaved fp8 ([pair p,
    slot s] = channel 2p+s) so contraction is 256-wide at 0.5 cycles/row.
    gn_w is folded into the fp8 weights host-side; the device folds only
    the per-group rstd (constant within a pair -> per-partition
    tensor_scalar). PE busy ~117us -> ~93us.
  - E[x^2] for the GroupNorm variance is estimated from half the columns
    (var err ~1%, far below fp8 quantization noise): halves the square
    cost on DVE.
  - Prep restructured into two phases: (A) stats+affine using the wide
    stats PSUM, (B) every projection through a 6-deep 1-bank PSUM ring
    with PSUM->SBUF casts alternating ScalarE/DVE and remap DMAs chunked,
    so item 0 starts ~20us in (was ~45). Input DMAs spread across the
    SP/Act/Pool queues.
  - softmax exp splits ScalarE (exact, ~19/32 chunks) / DVE (one-op
    Schraudolph e5m2 bit trick). GPSIMD cannot access PSUM on TRN2, so it
    only carries SBUF-side work (ones-memsets, weight scaling, d_x).
  - out-pairs lag their exp chunks by ~4 e-chunks so a slow exp never
    head-of-line-blocks PE; divide emitted before the next item's DVE
    exps to release the single pout PSUM slot early.
  - x ships twice: bf16 own-half for the residual (1MB) + fp8
    pair-interleaved full-width for Q/stats (1MB); ctx only as fp8 (1MB).

Simulated (instruction_cost_v2 timeline): 220us/iteration; HW rel err
5.8e-3 vs the f32 reference (gate 2e-2).
"""

import sys

if "/opt/trn_rl_repo" not in sys.path:
    sys.path.insert(0, "/opt/trn_rl_repo")

import copy
from contextlib import ExitStack

import numpy as np
import ml_dtypes

import bass_rust
import concourse.bass as bass
import concourse.mybir as mybir
import concourse.tile as tile
from concourse.bass_utils import run_bass_kernel_spmd
from concourse.vector_clock import ScopedClock

BF16 = ml_dtypes.bfloat16
F32 = mybir.dt.float32
BF = mybir.dt.bfloat16
F8 = mybir.dt.float8e5
F8E4 = mybir.dt.float8e4
DR = mybir.MatmulPerfMode.DoubleRow

N_CORES = 8
B, C, HW = 4, 256, 4096
HALF = HW // 2          # spatial columns per core
NH, HD = 4, 64          # heads, head dim
P = 128                 # partitions
NSUB = C // P           # channel subtiles (2)
GROUPS = 32             # groupnorm groups
CH_PER_G = C // GROUPS  # 8
GN_N = CH_PER_G * HW    # elements per group (32768)
EPS = 1e-5
DJ = 1024               # main-loop d-chunk (exp granularity)
NDJ = HALF // DJ        # 2
NE = HW // P            # 32 e-chunks
ALU = mybir.AluOpType
ACTF = mybir.ActivationFunctionType
AXX = mybir.AxisListType.X

# e5m2 Schraudolph exp2 bit trick: bits = round(4*log2e*S/8 + B) as uint8
# viewed as fp8e5. |S| <= ~58 so bits stay in [18, 102] -- far from both
# uint8 wrap and e5m2 inf (124).
EXPA = 0.125 * 4.0 * 1.4426950408889634
EXPB = 59.8
EXPB_POOL = 59.8        # adjusted if Pool's f32->uint8 rounding differs

# exp engine per e-chunk (ec 0..31):
# A = exact exp on ScalarE, D = Schraudolph on DVE. Counts tuned so both
# engines land ~165us busy per iteration under instruction_cost_v2
# (DVE pays the PSUM-read penalty; Act is fastest at ~1.04us/chunk).


def _mkpat(ca, cd):
    cnt = {"A": ca, "D": cd}
    acc = {"A": 0.0, "D": 0.0}
    pat = []
    for _ in range(ca + cd):
        for e in acc:
            acc[e] += cnt[e]
        pick = max(sorted(acc), key=lambda e: acc[e])
        acc[pick] -= ca + cd
        pat.append(pick)
    return tuple(pat)


# per-item (A, D) chunk counts: GPSIMD cannot touch PSUM on TRN2, so the
# softmax exp runs only on ScalarE (exact) + DVE (Schraudolph); items 4/5
# shift exp work off DVE (which drains the dj=0 Wo output chunks there).
ITEM_PAT = [(19, 13), (18, 14), (19, 13), (18, 14),
            (21, 11), (21, 11), (19, 13), (18, 14)]


def _fix_pat(pat):
    # ec 1 and 2 are emitted right after the previous item's divide on DVE;
    # a DVE exp there would queue behind it and stall the pipeline start.
    pat = list(pat)
    for pos in (1, 2):
        if pat[pos] == "D":
            for j in range(3, len(pat)):
                if pat[j] != "D":
                    pat[pos], pat[j] = pat[j], pat[pos]
                    break
    return tuple(pat)


EXP_PATS = [_fix_pat(_mkpat(*c)) for c in ITEM_PAT]

SPLIT_FOR_WALRUS = True  # sim disables: CoreSim rejects raw SyncInfo


class SplitDrainTileContext(tile.TileContext):
    """TileContext whose exit drain splits sem waits across multiple Drain
    instructions — the walrus build in this container rejects >2 sync waits
    on a single Drain ("Too many sync wait commands")."""

    def _drain_and_barrier(self, tick_clock, wait_clock):
        if not SPLIT_FOR_WALRUS:
            return super()._drain_and_barrier(tick_clock, wait_clock)
        drain_inst = self.nc.sync.drain()
        wait_clock.add_sem_waits(
            drain_inst.ins, ScopedClock({None: tick_clock.global_clock})
        )
        si = drain_inst.ins.sync_info
        if si is not None and si.on_wait and len(si.on_wait) > 1:
            waits = list(si.on_wait)
            si.on_wait = waits[:1]
            drain_inst.ins.sync_info = si
            for w in waits[1:]:
                extra = self.nc.sync.drain()
                extra.ins.sync_info = bass_rust.SyncInfo(on_wait=[w], on_update=[])
        self.nc.all_engine_barrier()
        popped = self.nc._tile_sem_poison_stack.pop()
        assert popped is self._sem_poison
        self.nc.clear_and_free_semaphores(list(self.sems.allocated().values()))
        self.nc.all_engine_barrier()


_NOP_TMPL = []


def _nop_template():
    if not _NOP_TMPL:
        tb = bass.Bass()
        with tb.bb("t"):
            _NOP_TMPL.append(copy.copy(tb.vector.nop().ins))
    return _NOP_TMPL[0]


def _split_excess_waits(nc, limit=1):
    """This container's walrus rejects instructions carrying more than ~2
    sync-wait commands. Spill excess waits onto same-engine NoOps inserted
    just before the overloaded instruction (waiting earlier on the same
    engine is semantics-preserving; NoOps have no dependents, so no cycles
    can form)."""
    tmpl = _nop_template()
    n = 0

    def fix(blk):
        nonlocal n
        if hasattr(blk, "instructions"):
            out = []
            changed = False
            for inst in blk.instructions:
                si = inst.sync_info
                ow = list(si.on_wait) if (si is not None and si.on_wait) else []
                lim = 1 if ("DMA" in inst.opcode or inst.opcode == "Drain") \
                    else limit
                if len(ow) > lim:
                    changed = True
                    for w in ow[:-lim]:
                        sp = copy.copy(tmpl)
                        n += 1
                        sp.name = f"I-wsp-{n}"
                        sp.engine = inst.engine
                        sp.sync_info = bass_rust.SyncInfo(on_wait=[w],
                                                          on_update=[])
                        out.append(sp)
                    si.on_wait = ow[-lim:]
                    inst.sync_info = si
                out.append(inst)
            if changed:
                blk.instructions = out
        for sub in getattr(blk, "blocks", []) or []:
            fix(sub)

    for f in nc.m.functions:
        for blk in f.blocks:
            fix(blk)
    return n


def build_module(loop: int = 1) -> bass.Bass:
    """loop > 1 emits the kernel body that many times back-to-back (each in
    its own tile context, so semaphores reset between iterations). Used for
    on-device loop timing: the marginal cost of +1 iteration is the true HW
    exec time, with all per-call dispatch/transfer overhead differenced out."""
    nc = bass.Bass()

    dr = {}
    dr["xb"] = nc.dram_tensor("xb", [C, HALF], BF, kind="ExternalInput")
    dr["x8"] = nc.dram_tensor("x8", [P, 2, HW], F8E4, kind="ExternalInput")
    dr["cb8"] = nc.dram_tensor("cb8", [P, 2, HW], F8E4, kind="ExternalInput")
    dr["wq8"] = nc.dram_tensor("wq8", [P, 2, C], F8E4, kind="ExternalInput")
    dr["wk8"] = nc.dram_tensor("wk8", [P, 2, C], F8E4, kind="ExternalInput")
    dr["wv8"] = nc.dram_tensor("wv8", [P, 2, C], F8E4, kind="ExternalInput")
    dr["wqt"] = nc.dram_tensor("wqt", [C, C], BF, kind="ExternalInput")
    dr["wot"] = nc.dram_tensor("wot", [C, C], BF, kind="ExternalInput")
    dr["gnp"] = nc.dram_tensor("gnp", [P, NSUB, 2], F32, kind="ExternalInput")
    dr["bo"] = nc.dram_tensor("bo", [P, NSUB], F32, kind="ExternalInput")
    dr["gsel8"] = nc.dram_tensor("gsel8", [P, 2, GROUPS], F8E4,
                                 kind="ExternalInput")
    dr["selt8"] = nc.dram_tensor("selt8", [GROUPS, P], F32,
                                 kind="ExternalInput")
    dr["seltn"] = nc.dram_tensor("seltn", [GROUPS, 2, P], F32,
                                 kind="ExternalInput")
    dr["out"] = nc.dram_tensor("out", [C, HALF], BF, kind="ExternalOutput")

    for _ in range(loop):
        with SplitDrainTileContext(nc) as tc:
            _emit(nc, tc, dr)
    if SPLIT_FOR_WALRUS:
        _split_excess_waits(nc)
    return nc


def _emit(nc, tc, dr):
    with ExitStack() as ctx:
        pw = ctx.enter_context(tc.tile_pool(name="pw", bufs=1))
        pmain = ctx.enter_context(tc.tile_pool(name="pmain", bufs=1))
        ptp = ctx.enter_context(tc.tile_pool(name="ptp", bufs=4))
        psmall = ctx.enter_context(tc.tile_pool(name="psmall", bufs=2))

        # ---- tiles ----
        wq8_sb = pw.tile([P, 2, C], F8E4, name="wq8_sb")
        wk8_sb = pw.tile([P, 2, C], F8E4, name="wk8_sb")
        wv8_sb = pw.tile([P, 2, C], F8E4, name="wv8_sb")
        wqt_sb = pw.tile([P, NSUB, C], BF, name="wqt_sb")
        wo_sb = pw.tile([P, NSUB, C], BF, name="wo_sb")
        gnp_sb = pw.tile([P, NSUB, 2], F32, name="gnp_sb")
        bo_sb = pw.tile([P, NSUB], F32, name="bo_sb")
        gsel8_sb = pw.tile([P, 2, GROUPS], F8E4, name="gsel8_sb")
        selt8_sb = pw.tile([GROUPS, P], F32, name="selt8_sb")
        seltn_sb = pw.tile([GROUPS, 2, P], F32, name="seltn_sb")
        eps_sb = pw.tile([GROUPS, 1], F32, name="eps_sb")
        nc.vector.memset(eps_sb[:], EPS)

        xb_sb = pmain.tile([P, NSUB, HALF], BF, name="xb_sb")
        x8_sb = pmain.tile([P, 2, HW], F8E4, name="x8_sb")
        cb8_sb = pmain.tile([P, 2, HW], F8E4, name="cb8_sb")
        q_sb = pmain.tile([P, NSUB, HALF], F8E4, name="q_sb")
        k_sb = pmain.tile([P, NSUB, HW], F8E4, name="k_sb")
        q8_sb = [pmain.tile([HD, 2, HALF], F8E4, name=f"q8_sb{i}")
                 for i in range(NSUB)]
        k8_sb = [pmain.tile([HD, 2, HW], F8E4, name=f"k8_sb{i}")
                 for i in range(NSUB)]
        vt8_sb = pmain.tile([P, NE // 2, 2, NH, 2 * HD], F8, name="vt8_sb")
        ao_sb = pmain.tile([P, NSUB, HALF], BF, name="ao_sb")
        stats_sb = pmain.tile([GROUPS, 4], F32, name="stats_sb")
        wqs8_sb = pmain.tile([P, 2, C], F8E4, name="wqs8_sb")
        wks8_sb = pmain.tile([P, 2, C], F8E4, name="wks8_sb")
        qb_sb = pmain.tile([P, NSUB], F32, name="qb_sb")

        # ---- input DMAs. Each engine queue is serial; spread the loads so
        # the SP queue carries only ctx + the critical k8/q8 remaps ----
        NJ = 4
        CHK = HW // NJ  # 1024
        nc.sync.dma_start(gsel8_sb[:], dr["gsel8"][:])
        for j in range(NJ):
            sl = slice(j * CHK, (j + 1) * CHK)
            nc.sync.dma_start(cb8_sb[:, :, sl], dr["cb8"][:, :, sl])
        nc.sync.dma_start(wk8_sb[:], dr["wk8"][:])
        for j in range(NJ):
            sl = slice(j * CHK, (j + 1) * CHK)
            nc.scalar.dma_start(x8_sb[:, :, sl], dr["x8"][:, :, sl])
        nc.gpsimd.dma_start(selt8_sb[:], dr["selt8"][:])
        nc.gpsimd.dma_start(seltn_sb[:], dr["seltn"][:])
        nc.gpsimd.dma_start(gnp_sb[:], dr["gnp"][:])
        nc.scalar.dma_start(wq8_sb[:], dr["wq8"][:])
        nc.scalar.dma_start(
            wqt_sb[:], dr["wqt"][:].rearrange("(t p) o -> p t o", p=P))
        nc.scalar.dma_start(wv8_sb[:], dr["wv8"][:])
        # ones-columns of v^T (denominator rows), first half: Pool is idle
        # at the start (a strided DMA would need 16k descriptors)
        nc.gpsimd.memset(vt8_sb[:, 0:NE // 4, :, :, HD:2 * HD], 1.0)

        # ============ prep: GN stats -> folded weights -> Q/K/V^T ==========
        # Everything before the attention loop lives here, spread across
        # engines: squares on DVE, small affine math on Pool, rsqrt on Act,
        # PSUM->SBUF casts round-robin, all selector/projection matmuls on
        # PE, remap DMAs chunked so item 0 can start as soon as its first
        # q8/k8 chunks land.
        with ExitStack() as prep:
            pps = prep.enter_context(
                tc.tile_pool(name="pps", bufs=4, space="PSUM"))
            pmm = prep.enter_context(
                tc.tile_pool(name="pmm", bufs=1, space="PSUM"))
            pchunk = prep.enter_context(tc.tile_pool(name="pchunk", bufs=2))

            SQ_JS = (0, 2)  # E[x^2] from half the columns: var err ~1%,
            # far below the fp8 quantization noise; halves the sq cost

            def stats_mms(src_sb, tensor_idx):
                """Per-group sums of src (all columns) and src^2 (half the
                columns) via the pair-space fp8 selector matmul (DoubleRow:
                256-wide contraction), streamed in NJ chunks."""
                ps = {
                    kind: pps.tile([GROUPS, 512], F32, tag="stat",
                                   name=f"ps_st{tensor_idx}{kind}")
                    for kind in range(2)
                }
                for j in range(NJ):
                    sl = slice(j * CHK, (j + 1) * CHK)
                    if j in SQ_JS:
                        sq = pchunk.tile([P, 2, CHK], F8E4, tag="sq",
                                         name="sq")
                        nc.vector.tensor_mul(sq[:], src_sb[:, :, sl],
                                             src_sb[:, :, sl])
                    for half in range(CHK // 512):
                        s2 = slice(j * CHK + half * 512,
                                   j * CHK + (half + 1) * 512)
                        s2q = slice(half * 512, (half + 1) * 512)
                        first = j == 0 and half == 0
                        last = j == NJ - 1 and half == CHK // 512 - 1
                        nc.tensor.matmul(ps[0][:], gsel8_sb[:],
                                         src_sb[:, :, s2],
                                         start=first, stop=last,
                                         perf_mode=DR)
                        if j in SQ_JS:
                            nc.tensor.matmul(
                                ps[1][:], gsel8_sb[:], sq[:, :, s2q],
                                start=(j == SQ_JS[0] and half == 0),
                                stop=(j == SQ_JS[-1] and
                                      half == CHK // 512 - 1),
                                perf_mode=DR)
                for kind in range(2):
                    col = 2 * tensor_idx + kind
                    nc.vector.reduce_sum(stats_sb[:, col:col + 1],
                                         ps[kind][:], axis=AXX)

            def affine(tensor_idx, rstd8_out):
                """stats columns -> per-group mean/rstd [32, 1]; rstd
                expanded to pair space [P, 1] via the fp32 selector."""
                inv_n = 1.0 / GN_N
                inv_n2 = float(NJ) / len(SQ_JS) / GN_N
                col = 2 * tensor_idx
                mean = psmall.tile([GROUPS, 1], F32, tag="mn",
                                   name=f"mean{tensor_idx}")
                rstd = psmall.tile([GROUPS, 1], F32, tag="rs",
                                   name=f"rstd{tensor_idx}")
                nc.gpsimd.tensor_scalar_mul(
                    mean[:], stats_sb[:, col:col + 1], inv_n)
                nc.gpsimd.tensor_scalar_mul(
                    rstd[:], stats_sb[:, col + 1:col + 2], inv_n2)
                m2 = psmall.tile([GROUPS, 1], F32, tag="m2", name="m2")
                nc.gpsimd.tensor_mul(m2[:], mean[:], mean[:])
                nc.gpsimd.tensor_sub(rstd[:], rstd[:], m2[:])
                nc.scalar.activation(rstd[:], rstd[:], ACTF.Sqrt,
                                     bias=eps_sb[:])
                nc.vector.reciprocal(rstd[:], rstd[:])
                psg = pmm.tile([P, 512], F32, tag="mm",
                               name=f"psg{tensor_idx}")
                nc.tensor.matmul(psg[:, :1], selt8_sb[:], rstd[:],
                                 start=True, stop=True)
                nc.vector.tensor_copy(rstd8_out[:], psg[:, :1])
                return mean, rstd

            # --- K-side stats, then x stats back-to-back (phase A owns
            # the wide stats PSUM; projections run in phase B with a
            # 6-deep 1-bank ring) ---
            stats_mms(cb8_sb, 0)
            rstd8_c = psmall.tile([P, 1], F32, tag="r8", name="rstd8_c")
            affine(0, rstd8_c)
            nc.gpsimd.tensor_scalar(wks8_sb[:], wk8_sb[:], rstd8_c[:, 0:1],
                                    None, op0=ALU.mult)
            stats_mms(x8_sb, 1)
            rstd8_x = psmall.tile([P, 1], F32, tag="r8", name="rstd8_x")
            mean_x, rstd_x = affine(1, rstd8_x)
            nc.gpsimd.tensor_scalar(wqs8_sb[:], wq8_sb[:], rstd8_x[:, 0:1],
                                    None, op0=ALU.mult)
            # normal-space (mean, rstd) per channel for the folded q bias:
            # d_x[c] = b[c] - mean[g(c)]*w[c]*rstd[g(c)]. The subtile mask
            # lives in the two host-side selector matrices (partition slices
            # must be 32-aligned, so no on-device masking).
            rhs32 = psmall.tile([GROUPS, 2], F32, tag="r4", name="rhs32")
            nc.gpsimd.tensor_copy(rhs32[:, 0:1], mean_x[:])
            nc.gpsimd.tensor_copy(rhs32[:, 1:2], rstd_x[:])
            psn = pmm.tile([P, 512], F32, tag="mm", name="psn")
            for t in range(NSUB):
                nc.tensor.matmul(psn[:, 2 * t:2 * t + 2], seltn_sb[:, t, :],
                                 rhs32[:], start=True, stop=True)
            psn_s = psmall.tile([P, 4], F32, tag="sc2", name="psn_s")
            nc.vector.tensor_copy(psn_s[:], psn[:, :4])
            scal = psmall.tile([P, NSUB], F32, tag="sc", name="scal")
            for t in range(NSUB):
                nc.vector.tensor_mul(scal[:, t:t + 1],
                                     psn_s[:, 2 * t:2 * t + 1],
                                     psn_s[:, 2 * t + 1:2 * t + 2])
            tmpd = psmall.tile([P, NSUB], F32, tag="tm", name="tmpdx")
            nc.gpsimd.tensor_mul(tmpd[:], scal[:], gnp_sb[:, :, 0])
            d_x = psmall.tile([P, NSUB], BF, tag="dx", name="d_x")
            nc.gpsimd.tensor_sub(d_x[:], gnp_sb[:, :, 1], tmpd[:])

            # --- q bias (needs phase A's pmm) ---
            psb = pmm.tile([P, 512], F32, tag="mm", name="psb")
            for i in range(NSUB):
                for t in range(NSUB):
                    nc.tensor.matmul(psb[:, 256 * i:256 * i + 1],
                                     wqt_sb[:, t, i * P:(i + 1) * P],
                                     d_x[:, t:t + 1],
                                     start=(t == 0), stop=(t == NSUB - 1))
            nc.vector.tensor_copy(qb_sb[:, 0:1], psb[:, 0:1])
            nc.vector.tensor_copy(qb_sb[:, 1:2], psb[:, 256:257])

        # ===== prep phase B: all projections through a 6-deep PSUM ring ====
        with ExitStack() as prepb:
            pkp = prepb.enter_context(
                tc.tile_pool(name="pkp", bufs=6, space="PSUM"))

            def k_proj_chunk(i, jd):
                psk = pkp.tile([P, 512], F32, tag="kp", name="psk")
                nc.tensor.matmul(psk[:], wks8_sb[:, :, i * P:(i + 1) * P],
                                 cb8_sb[:, :, jd * 512:(jd + 1) * 512],
                                 start=True, stop=True, perf_mode=DR)
                if jd % 2 == 1:
                    nc.vector.tensor_copy(
                        k_sb[:, i, jd * 512:(jd + 1) * 512], psk[:])
                else:
                    nc.scalar.copy(
                        k_sb[:, i, jd * 512:(jd + 1) * 512], psk[:])

            def vt_proj(ec):
                psv = pkp.tile([P, 512], F32, tag="kp", name="psv")
                nc.tensor.matmul(psv[:, :C],
                                 cb8_sb[:, :, ec * P:(ec + 1) * P],
                                 wv8_sb[:], start=True, stop=True,
                                 perf_mode=DR)
                dst = vt8_sb[:, ec // 2, ec % 2, :, 0:HD]
                srcv = psv[:, :C].rearrange("p (h c) -> p h c", c=HD)
                if ec % 2 == 1:
                    nc.vector.tensor_copy(dst, srcv)
                else:
                    nc.scalar.copy(dst, srcv)

            for jd in range(HW // 512):
                k_proj_chunk(0, jd)
                if jd % 2 == 1:
                    e0 = (jd - 1) * 512
                    nc.sync.dma_start(k8_sb[0][:, :, e0:e0 + 1024],
                                      k_sb[:, 0, e0:e0 + 1024])
            # partition remap (channel c -> partition 32h' + c//2, slot c%2)
            # for DoubleRow: SBUF->SBUF DMA, element orders match exactly
            for i in range(NSUB):
                for jd in range(HALF // 512):
                    psq = pkp.tile([P, 512], F32, tag="kp", name="psq")
                    nc.tensor.matmul(psq[:],
                                     wqs8_sb[:, :, i * P:(i + 1) * P],
                                     x8_sb[:, :, jd * 512:(jd + 1) * 512],
                                     start=True, stop=True, perf_mode=DR)
                    sl = slice(jd * 512, (jd + 1) * 512)
                    if jd % 2 == 0:
                        nc.vector.tensor_scalar(
                            q_sb[:, i, sl], psq[:], qb_sb[:, i:i + 1],
                            None, op0=ALU.add)
                    else:
                        nc.scalar.activation(q_sb[:, i, sl], psq[:],
                                             ACTF.Identity,
                                             bias=qb_sb[:, i:i + 1])
                    nc.sync.dma_start(q8_sb[i][:, :, sl], q_sb[:, i, sl])
            for ec in range(8):
                vt_proj(ec)
            # second half of the v^T ones rows (pairs 8-15, first consumed
            # ~two-thirds into item 0)
            nc.gpsimd.memset(vt8_sb[:, NE // 4:NE // 2, :, :, HD:2 * HD],
                             1.0)

            # --- K subtile 1 (needed from item 2 on) + remaining V^T ---
            for jd in range(HW // 512):
                k_proj_chunk(1, jd)
                if jd % 2 == 1:
                    e0 = (jd - 1) * 512
                    nc.sync.dma_start(k8_sb[1][:, :, e0:e0 + 1024],
                                      k_sb[:, 1, e0:e0 + 1024])
            for ec in range(8, NE):
                vt_proj(ec)
            # late-needed loads, last on the SP queue: Wo/bo/xb are first
            # read at the first Wo output chunk (item 4)
            nc.sync.dma_start(
                wo_sb[:], dr["wot"][:].rearrange("(t p) o -> p t o", p=P))
            nc.sync.dma_start(bo_sb[:], dr["bo"][:])
            nc.sync.dma_start(
                xb_sb[:], dr["xb"][:].rearrange("(t p) d -> p t d", p=P))

        # ================= attention main pipeline =================
        pst = ctx.enter_context(tc.tile_pool(name="psum_st", bufs=3,
                                             space="PSUM"))
        pout = ctx.enter_context(tc.tile_pool(name="psum_out", bufs=1,
                                              space="PSUM"))

        items = [(dj, h) for dj in range(NDJ) for h in range(NH)]
        state = {}   # per-item: po tile, pts list
        wo_queue = []
        NPAIR = NE // 2

        def emit_st_exp(n, ec):
            dj, h = items[n]
            hs = h // 2
            d0 = dj * DJ
            m, half = ec // 2, ec % 2
            st = pst.tile([P, DJ], F32, tag="st", name="st")
            hb = 32 * (h % 2)
            lhsT = k8_sb[hs][hb:hb + 32, :, ec * P:(ec + 1) * P]
            for s in range(DJ // 512):
                nc.tensor.matmul(
                    st[:, s * 512:(s + 1) * 512], lhsT,
                    q8_sb[hs][hb:hb + 32, :,
                              d0 + s * 512:d0 + (s + 1) * 512],
                    start=True, stop=True, perf_mode=DR)
            if half == 0:
                pt = ptp.tile([P, 2, DJ], F8, tag="ptq", bufs=6, name="ptq")
                state[n]["pts"].append(pt)
            pt = state[n]["pts"][m]
            eng = EXP_PATS[n][ec]
            if eng == "A":
                nc.scalar.activation(pt[:, half], st[:], ACTF.Exp,
                                     scale=0.125)
            else:
                nc.vector.tensor_scalar(pt[:, half].bitcast(mybir.dt.uint8),
                                        st[:], EXPA, EXPB,
                                        op0=ALU.mult, op1=ALU.add)

        def emit_out_pair(n, m):
            dj, h = items[n]
            po = state[n]["po"]
            pt = state[n]["pts"][m]
            vl = vt8_sb[:, m, :, h, :]
            for s in range(DJ // 512):
                nc.tensor.matmul(
                    po[:, s * 512:(s + 1) * 512], vl,
                    pt[:, :, s * 512:(s + 1) * 512],
                    start=(m == 0), stop=(m == NPAIR - 1),
                    perf_mode=DR)

        def emit_divide(n):
            dj, h = items[n]
            pb = (h % 2) * HD
            hs = h // 2
            d0 = dj * DJ
            po = state[n]["po"]
            # two PSUM operands per instruction are not allowed: reciprocal
            # into SBUF first, then multiply (PSUM x SBUF)
            rc = psmall.tile([HD, DJ], F32, tag="rc", name="rc")
            nc.vector.reciprocal(rc[:], po[HD:2 * HD, :])
            nc.vector.tensor_mul(ao_sb[pb:pb + HD, hs, d0:d0 + DJ],
                                 po[0:HD, :], rc[:])
            if h == NH - 1:
                for i in range(NSUB):
                    for s in range(DJ // 512):
                        wo_queue.append((dj, i, s))

        def emit_wo_chunk(eng=None):
            # Wo psum borrows a slot from the st pool (PSUM is fully booked:
            # st 3x2 + po 1x2 banks); the brief rotation wait is absorbed by
            # the exp engines' slack.
            dj, i, s = wo_queue.pop(0)
            sl = slice(dj * DJ + s * 512, dj * DJ + (s + 1) * 512)
            pso = pst.tile([P, DJ], F32, tag="st", name="pso")
            for t in range(NSUB):
                nc.tensor.matmul(
                    pso[:, :512], wo_sb[:, t, i * P:(i + 1) * P],
                    ao_sb[:, t, sl],
                    start=(t == 0), stop=(t == NSUB - 1))
            ot = psmall.tile([P, 512], BF, tag="ot", bufs=3, name="ot")
            (eng or nc.vector).scalar_tensor_tensor(
                ot[:], pso[:, :512], bo_sb[:, i:i + 1], xb_sb[:, i, sl],
                op0=ALU.add, op1=ALU.add)
            nc.sync.dma_start(
                dr["out"][:].rearrange("(t p) d -> p t d", p=P)[:, i, sl],
                ot[:])

        for n in range(len(items)):
            state[n] = {"po": None, "pts": []}
            for ec in range(NE):
                if ec == 0:
                    emit_st_exp(n, 0)
                    continue
                if ec == 1 and n > 0:
                    # finish the previous item BEFORE this head's second
                    # chunk: the divide lands ahead of this item's DVE/Pool
                    # exps in program order, releasing the single pout slot
                    # earlier so PE's first out-matmul does not stall
                    emit_out_pair(n - 1, NPAIR - 2)
                    emit_out_pair(n - 1, NPAIR - 1)
                    emit_divide(n - 1)
                    state[n - 1] = None
                emit_st_exp(n, ec)
                if ec == 2:
                    state[n]["po"] = pout.tile([P, DJ], F32, tag="po",
                                               name="po")
                # out-pair for pt[m] lags its exps by ~4 e-chunks so a slow
                # exp never head-of-line-blocks PE's in-order queue
                if ec >= 5 and ec % 2 == 1:
                    emit_out_pair(n, ec // 2 - 2)
                if wo_queue and ec % 4 == 0:
                    emit_wo_chunk()
        n = len(items) - 1
        emit_out_pair(n, NPAIR - 2)
        emit_out_pair(n, NPAIR - 1)
        emit_divide(n)
        while wo_queue:
            emit_wo_chunk(nc.vector)


_CACHE = {}


def _get_module():
    if "nc" not in _CACHE:
        _CACHE["nc"] = build_module()
    return _CACHE["nc"]


def make_in_maps(inputs):
    E4np = mybir.dt.np(F8E4)
    x = np.ascontiguousarray(
        np.asarray(inputs["x"], np.float32).reshape(B, C, HW))
    cx = np.ascontiguousarray(
        np.asarray(inputs["context"], np.float32).reshape(B, C, HW))
    Wq = np.asarray(inputs["Wq"], np.float32)
    Wk = np.asarray(inputs["Wk"], np.float32)
    Wv = np.asarray(inputs["Wv"], np.float32)
    Wo = np.asarray(inputs["Wo"], np.float32)
    bo = np.asarray(inputs["bo"], np.float32)
    gq_w = np.asarray(inputs["gn_q_w"], np.float32)
    gq_b = np.asarray(inputs["gn_q_b"], np.float32)
    gc_w = np.asarray(inputs["gn_ctx_w"], np.float32)

    # pair-interleaved fp8 weights ([p, s, o] = W[o, 2p+s]); gn weight
    # folded host-side into the q/k weights (the per-group rstd is folded
    # on-device: constant within a pair)
    wq8 = np.ascontiguousarray(
        (Wq * gq_w[None, :]).T.reshape(P, 2, C)).astype(E4np)
    wk8 = np.ascontiguousarray(
        (Wk * gc_w[None, :]).T.reshape(P, 2, C)).astype(E4np)
    wv8 = np.ascontiguousarray(Wv.T.reshape(P, 2, C)).astype(E4np)
    wqt = np.ascontiguousarray(Wq.T).astype(BF16)
    wot = np.ascontiguousarray(Wo.T).astype(BF16)
    # gnp columns: (w_x, b_x) per channel
    gnp = np.stack([gq_w, gq_b], axis=-1).reshape(NSUB, P, 2)
    gnp = np.ascontiguousarray(gnp.transpose(1, 0, 2))
    bo_t = np.ascontiguousarray(bo.reshape(NSUB, P).T)
    gsel8 = np.zeros((P, 2, GROUPS), E4np)
    for p in range(P):
        gsel8[p, :, p // (CH_PER_G // 2)] = 1
    selt8 = np.zeros((GROUPS, P), np.float32)
    for p in range(P):
        selt8[p // (CH_PER_G // 2), p] = 1
    seltn = np.zeros((GROUPS, NSUB, P), np.float32)
    for t in range(NSUB):
        for pn in range(P):
            seltn[16 * t + pn // 8, t, pn] = 1

    shared = dict(wq8=wq8, wk8=wk8, wv8=wv8, wqt=wqt, wot=wot, gnp=gnp,
                  bo=bo_t, gsel8=gsel8, selt8=selt8, seltn=seltn)
    in_maps = []
    for core in range(N_CORES):
        b, s = core // 2, core % 2
        m = dict(shared)
        # core's own spatial half first (stats are permutation-invariant)
        mine = x[b][:, s * HALF:(s + 1) * HALF]
        other = x[b][:, (1 - s) * HALF:(2 - s) * HALF]
        xr = np.concatenate([mine, other], axis=1)
        m["xb"] = np.ascontiguousarray(mine).astype(BF16)
        m["x8"] = np.ascontiguousarray(xr.reshape(P, 2, HW)).astype(E4np)
        m["cb8"] = np.ascontiguousarray(
            cx[b].reshape(P, 2, HW)).astype(E4np)
        in_maps.append(m)
    return in_maps


def assemble(results):
    outf = np.empty((B, C, HW), np.float32)
    for core in range(N_CORES):
        b, s = core // 2, core % 2
        outf[b][:, s * HALF:(s + 1) * HALF] = np.asarray(
            results[core]["out"], np.float32)
    return outf.reshape(B, C, 64, 64)


def kernel(**inputs) -> np.ndarray:
    nc = _get_module()
    in_maps = make_in_maps(inputs)
    res = run_bass_kernel_spmd(nc, in_maps, core_ids=list(range(N_CORES)))
    return assemble(res.results)
